# revision 11
# baseline (speedup 1.0000x reference)
"""Trainium2 Bass kernel for CTC loss (nn_CTCLayer).

Inputs (full, unsharded):
  y_true       [64, 48]  int32  labels (blank excluded)
  y_pred       [64, 128, 4000] float32 probabilities
  label_length [64, 1]  int32
Output: loss [64, 1] float32  (= tf.keras ctc_batch_cost, input_length == T)

Strategy (pure data parallelism, 8 examples per core on 8 cores):

The CTC forward DP over S = 2L+1 = 97 extended states only ever reads
y_pred at the (<= L+1) classes present in each example's extended label
sequence. The host gathers those classes while sharding: it builds a
per-core paug[s, t, b] bf16 tensor (state-expanded probabilities + EPS,
zeros on dead states, aux-row copies for repeated labels) so the device
starts the DP immediately after one 256 KB DMA - no on-chip gather or
expansion. The DMA is split into three t-chunks so the first DP rounds
overlap the bulk transfer.

The DP runs in the scaled probability domain with states on partitions
and examples on the free axis. To halve the serial depth and keep both
the PE and DVE busy, the forward recursion (t = 0..63) and the backward
recursion (t = 127..64) run as two interleaved chains that meet at
t* = 63, where  P(l|x) = sum_s alpha_t*[s] * beta_t*[s]:

    fwd:  U_t = (F^T @ U_{t-1}) * p[:, t, :]      (matmul -> multiply)
    bwd:  V_t = G_{t} * p[:, t, :];  G_{t-1} = Bw^T @ V_t   (multiply -> matmul)

F and Bw are static per-core [128,128] bf16 matrices with entries
+-kappa (kappa = 2048 keeps products in fp32 range; exactly
representable in bf16). One sum-renormalization per chain (factors
re-applied in log space at the end) bounds the remaining drift.

Rows 97..111 (fwd) and 112..127 (bwd) are auxiliary "W" rows that
correct the forbidden skip transition s-2 -> s when ext[s] == ext[s-2]
(adjacent repeated labels): aux row i tracks the would-be-forbidden
contribution for its example only (its paug row is a copy of the
relevant state's row; other examples' entries are zero), and the
transition matrix subtracts it where the skip is forbidden. The
cancellation is bit-exact because the aux row's matmul column is a copy
of the source state's column and its multiplier bits are identical.
Pathological inputs with more repeats than aux rows fall back to an
exact host computation.

Padding states s > 2*label_length never influence the result states
(transitions are monotone in s) and their paug rows are zero.
"""

import os
import sys
import math

import numpy as np

if "/opt/trn_rl_repo" not in sys.path:
    sys.path.insert(0, "/opt/trn_rl_repo")

# ---------------------------------------------------------------- constants
B, T, C, L = 64, 128, 4000, 48
S = 2 * L + 1            # 97 extended states
P = 128                  # partitions
RF = 15                  # fwd aux rows: partitions 97..111
RB = 16                  # bwd aux rows: partitions 112..127
RB_OFF = RF              # bwd aux offset from S
NCORES = 8
BSH = B // NCORES        # 8 examples per core
BLANK = C - 1
EPS = 1e-7               # keras backend epsilon (reference adds before log)
KAPPA = 2048.0
TSTAR = 63               # fwd covers t=0..63, bwd covers t=127..64
RENORM_F = (32, 63)   # 63: normalize U before the meet (product must not underflow)
RENORM_B = (95, 64)   # 64: normalize V in the last bwd round for the same reason
NRE = len(RENORM_F) + len(RENORM_B)

# fp32 consts [128, CW] column layout
COL_IM = 0               # [0:8]     fwd init mask
COL_EM = 8               # [8:16]    bwd init (end-state indicator incl aux copies)
COL_ONE = 16             # [16:17]   fp32 ones column (final sum)
COL_BR = 17              # [17:145]  row 0 = ones row (renorm broadcast)
CW = 145
# bf16 consts [128, 2P+1]: [0:128] F, [128:256] Bw, [256] ones col (renorm sums)
CBW = 2 * P + 1

_CACHE = {}


# ---------------------------------------------------------------- host tables
def _build_core_tables(y_true, y_pred, label_length):
    """Returns (paug [P, T*BSH] bf16, constf [P, CW] f32, constb [P, CBW] bf16,
    overflow: bool).

    paug[s, t, b] = y_pred[b, t, ext[b, s]] + EPS for live states (zero on
    dead states s > 2*ll and on other examples' aux rows), flattened to
    [P, T*BSH] with (t, b) free layout so the per-round slice
    paug[:, t*BSH:(t+1)*BSH] is contiguous."""
    import ml_dtypes
    n = y_true.shape[0]
    ll = label_length.reshape(-1).astype(np.int64)
    lab = np.where(np.arange(L)[None, :] < ll[:, None], y_true.astype(np.int64), BLANK)

    aug = []  # (i, b, s_i): repeat at odd state s_i (skip s_i-2 -> s_i forbidden)
    for b in range(n):
        for s_i in range(3, int(min(2 * ll[b] - 1, S - 1)) + 1, 2):
            j = (s_i - 1) // 2
            if lab[b, j] == lab[b, j - 1]:
                aug.append((len(aug), b, s_i))
    overflow = len(aug) > min(RF, RB)
    aug = aug[:min(RF, RB)]

    paug = np.zeros((P, T, n), dtype=ml_dtypes.bfloat16)
    for b in range(n):
        llb = int(ll[b])
        # gather the llb+1 distinct class rows once: [T, llb+1]
        cls = np.concatenate([lab[b, :llb], [BLANK]])
        rows = (y_pred[b][:, cls].astype(np.float32) + EPS).astype(ml_dtypes.bfloat16)
        for s in range(2 * llb + 1):
            k = llb if s % 2 == 0 else (s - 1) // 2
            paug[s, :, b] = rows[:, k]
        for (i, bb, s_i) in aug:
            if bb == b:
                j = (s_i - 1) // 2
                paug[S + i, :, b] = rows[:, j - 1]        # fwd aux: p[s_i - 2]
                paug[S + RB_OFF + i, :, b] = rows[:, j]   # bwd aux: p[s_i]

    # bake the chain-head masks into the t=0 / t=127 columns so round 1
    # feeds pa views straight into the matmuls (t=0 is only read by the
    # fwd init, t=127 only by the bwd init)
    im = np.zeros((P, n), dtype=bool)
    im[0, :] = im[1, :] = True
    em = np.zeros((P, n), dtype=bool)
    for b in range(n):
        em[2 * ll[b], b] = em[2 * ll[b] - 1, b] = True
    for (i, b, s_i) in aug:
        if s_i == 3:
            im[S + i, b] = True
        em[S + RB_OFF + i, b] = em[s_i, b]
    paug[:, 0, :] = np.where(im, paug[:, 0, :], ml_dtypes.bfloat16(0.0))
    paug[:, T - 1, :] = np.where(em, paug[:, T - 1, :], ml_dtypes.bfloat16(0.0))

    # forward lhsT: F[k, m] = kappa * allowed(k -> m)
    F = np.zeros((P, P), dtype=np.float64)
    for m in range(S):
        F[m, m] = 1.0
        if m >= 1:
            F[m - 1, m] = 1.0
        if m >= 2 and (m % 2 == 1):
            F[m - 2, m] = 1.0
    for (i, b, s_i) in aug:
        F[S + i, s_i] = -1.0
    for (i, b, s_i) in aug:
        F[:, S + i] = F[:, s_i - 2]

    # backward lhsT: Bw[k, m] = kappa * allowed(m -> k); G_{t-1} = Bw^T @ V_t,
    # V = G * p. Aux row i tracks V[s_i]; subtracted where the skip is forbidden.
    Bw = np.zeros((P, P), dtype=np.float64)
    for k in range(S):
        Bw[k, k] = 1.0
        if k >= 1:
            Bw[k, k - 1] = 1.0
        if k >= 2 and (k % 2 == 1):
            Bw[k, k - 2] = 1.0
    for (i, b, s_i) in aug:
        Bw[S + RB_OFF + i, s_i - 2] = -1.0
    for (i, b, s_i) in aug:
        Bw[:, S + RB_OFF + i] = Bw[:, s_i]

    one = ml_dtypes.bfloat16(1.0)
    constb = np.zeros((P, CBW), dtype=ml_dtypes.bfloat16)
    constb[:, 0:P] = (F * KAPPA).astype(ml_dtypes.bfloat16)
    constb[:, P:2 * P] = (Bw * KAPPA).astype(ml_dtypes.bfloat16)
    constb[:, 2 * P] = one

    constf = np.zeros((P, CW), dtype=np.float32)
    constf[0, COL_IM:COL_IM + BSH] = 1.0
    constf[1, COL_IM:COL_IM + BSH] = 1.0
    for (i, b, s_i) in aug:
        if s_i == 3:
            constf[S + i, COL_IM + b] = 1.0
    for b in range(n):
        constf[2 * ll[b], COL_EM + b] = 1.0
        constf[2 * ll[b] - 1, COL_EM + b] = 1.0
    for (i, b, s_i) in aug:
        constf[S + RB_OFF + i, COL_EM + b] = constf[s_i, COL_EM + b]
    constf[:, COL_ONE] = 1.0
    constf[0, COL_BR:COL_BR + P] = 1.0
    return paug.reshape(P, T * n), constf, constb, overflow


# ---------------------------------------------------------------- host fallback
def _host_ctc(y_true_b, y_pred_b, ll_b):
    """Exact log-domain port of the reference for one example (float64)."""
    NEG = -1e30
    ll = int(ll_b)
    lab = np.where(np.arange(L) < ll, y_true_b.astype(np.int64), BLANK)
    ext = np.full((S,), BLANK, dtype=np.int64)
    ext[1::2] = lab
    lp = np.log(y_pred_b.astype(np.float64) + EPS)[:, ext]    # [T, S]
    ext_m2 = np.concatenate([[BLANK, BLANK], ext[:-2]])
    allow = (ext != BLANK) & (ext != ext_m2)
    alpha = np.where(np.arange(S) < 2, lp[0], NEG)
    for t in range(1, T):
        a0 = alpha
        a1 = np.concatenate([[NEG], alpha[:-1]])
        a2 = np.where(allow, np.concatenate([[NEG, NEG], alpha[:-2]]), NEG)
        m = np.maximum(np.maximum(a0, a1), a2)
        alpha = m + np.log(np.exp(a0 - m) + np.exp(a1 - m) + np.exp(a2 - m)) + lp[t]
    ab, al = alpha[2 * ll], alpha[2 * ll - 1]
    m = max(ab, al)
    return -(m + math.log(math.exp(ab - m) + math.exp(al - m)))


# ---------------------------------------------------------------- bass program
def _build_program():
    import concourse.bacc as bacc
    import concourse.bass as bass
    import concourse.tile as tile
    import concourse.mybir as mybir

    nc = bacc.Bacc("TRN2", target_bir_lowering=False, debug=False,
                   enable_asserts=False, num_devices=NCORES, num_swdge_queues=1)
    paug_d = nc.dram_tensor("paug", [P, T * BSH], mybir.dt.bfloat16, kind="ExternalInput")
    cf_d = nc.dram_tensor("constf", [P, CW], mybir.dt.float32, kind="ExternalInput")
    cb_d = nc.dram_tensor("constb", [P, CBW], mybir.dt.bfloat16, kind="ExternalInput")
    loss_d = nc.dram_tensor("loss", [1, BSH], mybir.dt.float32, kind="ExternalOutput")

    fp32 = mybir.dt.float32
    bf16 = mybir.dt.bfloat16
    mult = mybir.AluOpType.mult

    with tile.TileContext(nc) as tc:
        with (
            tc.tile_pool(name="cpool", bufs=1) as cpool,
            tc.tile_pool(name="upool", bufs=2) as upool,
            tc.tile_pool(name="spool", bufs=1) as spool,
            tc.tile_pool(name="psf", bufs=2, space="PSUM") as psf,
            tc.tile_pool(name="psb", bufs=2, space="PSUM") as psb,
            tc.tile_pool(name="pss", bufs=1, space="PSUM") as pss,
        ):
            # spread DMA issue (~700ns each) across the two HW-DGE engines
            # (sync/scalar); gpsimd DMAs go through the slow software DGE.
            # The chain-head chunks and cb (LDWEIGHTS) gate round 1 - first.
            pa = cpool.tile([P, T * BSH], bf16, tag="pa")
            C1E = 32 * BSH
            C2S = 96 * BSH
            nc.sync.dma_start(pa[:, 0:C1E], paug_d[:, 0:C1E])
            nc.scalar.dma_start(pa[:, C2S:], paug_d[:, C2S:])
            cb = cpool.tile([P, CBW], bf16, tag="cb")
            nc.sync.dma_start(cb[:], cb_d[:])
            cf = cpool.tile([P, CW], fp32, tag="cf")
            nc.scalar.dma_start(cf[:], cf_d[:])
            nc.sync.dma_start(pa[:, C1E:C2S], paug_d[:, C1E:C2S])

            # preload the Ln activation table while the DMAs stream in
            # (input: the constant-1.0 column of cf)
            lnwarm = spool.tile([1, 1], fp32, tag="lnwarm")
            nc.scalar.activation(lnwarm[:], cf[0:1, COL_ONE:COL_ONE + 1],
                                 mybir.ActivationFunctionType.Ln)

            def pslice(t):
                return pa[:, t * BSH:(t + 1) * BSH]

            F_ap = cb[:, 0:P]
            Bw_ap = cb[:, P:2 * P]
            onesb = cb[:, 2 * P:2 * P + 1]
            norms = spool.tile([1, NRE * BSH], fp32, tag="norms")
            ri = 0

            def renorm(Z, Zprev):
                """Divide state Z (AP) by the column sum of Zprev (AP, the
                previous round's state, already in SBUF) - the sum matmul/
                recip/broadcast run off the serial chain; only the final
                multiply joins it. Any positive factor is exact bookkeeping:
                we log precisely the reciprocal we apply."""
                nonlocal ri
                nm = pss.tile([1, BSH], fp32, tag="sm")
                nc.tensor.matmul(nm[:], onesb, Zprev, start=True, stop=True)
                rrow = norms[0:1, ri * BSH:(ri + 1) * BSH]
                nc.vector.reciprocal(rrow, nm[:])
                bc = pss.tile([P, BSH], fp32, tag="bc")
                nc.tensor.matmul(bc[:], cf[0:1, COL_BR:COL_BR + P], rrow,
                                 start=True, stop=True)
                Z2 = upool.tile([P, BSH], bf16, tag="Z2")
                nc.vector.tensor_tensor(out=Z2[:], in0=Z, in1=bc[:], op=mult)
                ri += 1
                return Z2

            # chain heads: the pre-masked t=0 / t=127 pa columns feed the
            # round-1 matmuls directly (no init multiplies, cf off the
            # critical path until the first renorm)
            U = pslice(0)
            gp = None  # bwd chain state (PSUM); round 1 uses the pa view

            Vprev = None
            for r in range(1, TSTAR + 2):
                tf_ = r           # fwd timestep this round (valid while <= TSTAR)
                tb = T - r        # bwd multiply timestep this round (127..64)
                # bwd: V = G * p[tb]; G(psum) = Bw^T V
                if gp is None:
                    V = pslice(tb)
                else:
                    V = upool.tile([P, BSH], bf16, tag="V")
                    nc.vector.tensor_tensor(out=V[:], in0=gp[:], in1=pslice(tb),
                                            op=mult)
                    V = V[:]
                if tb in RENORM_B:
                    V = renorm(V, Vprev)[:]
                Vprev = V
                gp = psb.tile([P, BSH], fp32, tag="gp")
                nc.tensor.matmul(gp[:], Bw_ap, V, start=True, stop=True)
                # fwd: psum = F^T U; U = psum * p[tf]
                if tf_ <= TSTAR:
                    stp = psf.tile([P, BSH], fp32, tag="stp")
                    nc.tensor.matmul(stp[:], F_ap, U, start=True, stop=True)
                    Uprev = U
                    Ut = upool.tile([P, BSH], bf16, tag="U")
                    nc.vector.tensor_tensor(
                        out=Ut[:], in0=stp[:], in1=pslice(tf_), op=mult)
                    U = Ut[:]
                    if tf_ in RENORM_F:
                        U = renorm(U, Uprev)[:]

            # meet: fin[b] = sum_s U_63[s, b] * G_63[s, b] (aux cross-terms vanish:
            # U is zero on bwd-aux rows, G zero on fwd-aux rows)
            prod = spool.tile([P, BSH], fp32, tag="prod")
            nc.vector.tensor_tensor(out=prod[:], in0=U, in1=gp[:], op=mult)
            fin = pss.tile([1, BSH], fp32, tag="sm")
            nc.tensor.matmul(fin[:], cf[:, COL_ONE:COL_ONE + 1], prod[:],
                             start=True, stop=True)
            lnfin = spool.tile([1, BSH], fp32, tag="lnfin")
            nc.scalar.activation(lnfin[:], fin[:], mybir.ActivationFunctionType.Ln)
            lnrec = spool.tile([1, NRE * BSH], fp32, tag="lnrec")
            nc.scalar.activation(lnrec[:], norms[:], mybir.ActivationFunctionType.Ln)
            lnrsum = spool.tile([1, BSH], fp32, tag="lnrsum")
            nc.vector.reduce_sum(
                lnrsum[:],
                lnrec[0:1, :].rearrange("p (j b) -> p b j", j=NRE),
                axis=mybir.AxisListType.X)
            total = spool.tile([1, BSH], fp32, tag="total")
            nc.vector.tensor_tensor(out=total[:], in0=lnrsum[:], in1=lnfin[:],
                                    op=mybir.AluOpType.subtract)
            loss_row = spool.tile([1, BSH], fp32, tag="loss_row")
            nc.vector.tensor_scalar_add(loss_row[:], total[:],
                                        float((T - 1) * math.log(KAPPA)))
            nc.sync.dma_start(loss_d[:], loss_row[:])

    nc.compile()
    return nc


def _get_program():
    if "nc" not in _CACHE:
        _CACHE["nc"] = _build_program()
    return _CACHE["nc"]


# ---------------------------------------------------------------- entry point
def kernel(y_true: np.ndarray, y_pred: np.ndarray, label_length: np.ndarray) -> np.ndarray:
    from concourse.bass_utils import run_bass_kernel_spmd

    y_true = np.asarray(y_true)
    y_pred = np.asarray(y_pred, dtype=np.float32)
    label_length = np.asarray(label_length)
    assert y_true.shape == (B, L) and y_pred.shape == (B, T, C), (
        f"unexpected shapes {y_true.shape} {y_pred.shape}")

    in_maps = []
    fallback_cores = []
    for core in range(NCORES):
        sl = slice(core * BSH, (core + 1) * BSH)
        paug, constf, constb, overflow = _build_core_tables(
            y_true[sl], y_pred[sl], label_length[sl])
        if overflow:
            fallback_cores.append(core)
        in_maps.append({
            "paug": paug,
            "constf": constf,
            "constb": constb,
        })

    nc = _get_program()
    res = run_bass_kernel_spmd(
        nc, in_maps, core_ids=list(range(NCORES)),
        trace=bool(int(os.environ.get("CTC_TRACE", "0"))),
    )
    _CACHE["last_result"] = res

    loss = np.zeros((B, 1), dtype=np.float32)
    for core in range(NCORES):
        loss[core * BSH:(core + 1) * BSH, 0] = res.results[core]["loss"][0]

    for core in fallback_cores:  # more repeats than aux rows (pathological)
        for b in range(BSH):
            g = core * BSH + b
            loss[g, 0] = _host_ctc(y_true[g], y_pred[g], label_length.reshape(-1)[g])
    return loss
